# revision 10
# baseline (speedup 1.0000x reference)
"""AdaLayerNorm (ragged gather_csr + LayerNorm) Trainium2 Bass kernel.

Runs SPMD on 8 NeuronCores, data-parallel over the node dimension: each core
gets a contiguous 16384-row shard of `feat`, replicated affine weights, and
its segment end-offsets clipped to the local row range, so the gather_csr
expansion stays device-local (per the sharding hint).

Per-core graph, HBM-roofline oriented (the kernel is DMA-bound: 64 MB of
feat in/out per core at ~410 GB/s aggregate over 16 DMA engines):
- Chunked 2 MB DMAs in a p-major row layout: each of the 128 partitions holds
  8 consecutive rows, so every DMA descriptor moves one contiguous 16 KB span.
- Engine balance keeps every pointwise engine under the ~10.2 us/chunk DMA
  pace: LayerNorm stats via bn_stats + final multiply on DVE; the segment
  mask (is_lt) and stats-combine small-ops on GpSimd; sqrt/normalize on
  ScalarE (GPSIMD cannot touch PSUM, so the PSUM-reading multiply stays on
  DVE and is_lt moves to GpSimd instead).
- The per-node segment broadcast g[seg(i), :] is computed on-device as ONE
  accumulating TensorE matmul per 128-row tile against a 0/1 comparison
  matrix A[s, i] = [row_i < end_s] (built via is_lt against per-chunk
  thresholds precomputed on host) and telescoped row differences
  dg[s] = g[s] - g[s+1]:  sum_{s: end_s > row} dg[s] == g[seg(row)] exactly.
  dg is fp16 (the only surviving rounding, ~1.6e-3 relative).
- Normalize and multiply are IN-PLACE on the input tile (no separate output
  buffer), freeing SBUF for a 9-deep input pool so the input DMA stream never
  stalls; outputs stream per half-chunk to cut the drain tail.
- Affine weight loads use a (p k) row interleave so every descriptor is a
  contiguous 8 KB / 1 KB span instead of 512 small strided rows.
"""

import sys

sys.path.insert(0, "/opt/trn_rl_repo")

import os

import numpy as np

import concourse.bacc as bacc
import concourse.mybir as mybir
import concourse.tile as tile
from concourse.bass_utils import run_bass_kernel_spmd

dt = mybir.dt
AF = mybir.ActivationFunctionType
ALU = mybir.AluOpType

EPS = 1e-5
P = 128
N_CORES = 8
N, D_FULL, S_FULL = 131072, 512, 64
ROWS = N // N_CORES


def _install_profshim():
    """Best-effort NTFF profiling hook for trace runs (optional)."""
    try:
        import types

        import antenv

        if getattr(antenv, "axon_hooks", None) is not None:
            return
        sys.path.insert(0, "/root/.axon_site/trn_agent_boot")
        import trn_boot

        hook = trn_boot._ntff_profile_via_ctypes("/opt/axon/libaxon_pjrt.so")
        m = types.ModuleType("antenv.axon_hooks")
        state = {"hook": hook}
        m.set_axon_ntff_profile_hook = lambda h: state.__setitem__("hook", h)
        m.get_axon_ntff_profile_hook = lambda: state["hook"]
        sys.modules["antenv.axon_hooks"] = m
        antenv.axon_hooks = m
    except Exception:
        pass


def build_kernel(
    rows=16384,
    D=512,
    S=64,
    chunk_tiles=8,
    skew=2,
    prefetch=6,
    bufs_in=9,
    bufs_sgn=5,
    bufs_sts=5,
):
    """Build the per-core Bass graph. All 8 cores run this same graph."""
    J = chunk_tiles
    chunk_rows = P * J
    nchunks = rows // chunk_rows
    assert rows % chunk_rows == 0
    WD = D
    KCH = WD // P

    nc = bacc.Bacc("TRN2", target_bir_lowering=False, debug=False)
    feat = nc.declare_dram_parameter("feat", [rows, D], dt.float32, isOutput=False)
    gfT = nc.declare_dram_parameter("gfT", [WD, S], dt.float32, isOutput=False)
    WT = nc.declare_dram_parameter("WT", [WD, D], dt.float32, isOutput=False)
    bb = nc.declare_dram_parameter("b", [1, D], dt.float32, isOutput=False)
    ohi = nc.declare_dram_parameter("off_hi", [S, nchunks], dt.float32, isOutput=False)
    out = nc.declare_dram_parameter("out", [rows, D], dt.float32, isOutput=True)

    with tile.TileContext(nc) as tc:
        with (
            tc.tile_pool(name="const", bufs=1) as cst,
            tc.tile_pool(name="inb", bufs=bufs_in) as in_pool,
            tc.tile_pool(name="sgn", bufs=bufs_sgn) as sgn_pool,
            tc.tile_pool(name="sts", bufs=bufs_sts) as sts_pool,
            tc.tile_pool(name="ps", bufs=1, space="PSUM") as ps_pool,
        ):
            # First feat chunks queue ahead of all init DMAs (startup latency)
            feat_v = feat.ap().rearrange("(c p j) d -> c p j d", p=P, j=J)
            out_v = out.ap().rearrange("(c p j) d -> c p j d", p=P, j=J)
            ints = {}

            def dma_in(c):
                ints[c] = in_pool.tile(
                    [P, J, D], dt.float32, tag="int", name=f"int{c}"
                )
                nc.sync.dma_start(ints[c][:], feat_v[c])

            for c0 in range(min(prefetch, nchunks)):
                dma_in(c0)

            # ---------------- init: affine g = gf @ W.T + b ----------------
            # (p k) interleave: partition p holds WD-rows 4p..4p+3, one
            # contiguous span per partition per DMA descriptor.
            wt_sb = cst.tile([P, KCH, D], dt.float32)
            nc.sync.dma_start(wt_sb[:], WT.ap().rearrange("(p k) d -> p k d", k=KCH))
            gft_sb = cst.tile([P, KCH, S], dt.float32)
            nc.sync.dma_start(gft_sb[:], gfT.ap().rearrange("(p k) s -> p k s", k=KCH))
            b_sb = cst.tile([1, D], dt.float32)
            nc.sync.dma_start(b_sb[:], bb.ap())
            ohi_sb = cst.tile([S, nchunks], dt.float32)
            nc.sync.dma_start(ohi_sb[:], ohi.ap())

            ones1 = cst.tile([1, S], dt.float32)
            nc.vector.memset(ones1[:], 1.0)

            psg = ps_pool.tile([S, D], dt.float32, tag="ps")
            for k in range(KCH):
                nc.tensor.matmul(
                    psg[:],
                    gft_sb[:, k, :],
                    wt_sb[:, k, :],
                    start=(k == 0),
                    stop=False,
                )
            nc.tensor.matmul(psg[:], ones1[:], b_sb[:], start=False, stop=True)
            # telescoped differences: dg[s] = g[s] - g[s+1]  (dg[63] = g[63]);
            # sum_{s >= seg} dg[s] telescopes exactly to g[seg]
            gf32 = cst.tile([S, D], dt.float32)
            nc.scalar.activation(gf32[:], psg[:], AF.Copy)
            gsh = cst.tile([S, D], dt.float32)
            nc.vector.memset(gsh[:], 0.0)
            nc.sync.dma_start(gsh[0 : S - 1, :], gf32[1:S, :])
            dg16 = cst.tile([S, D], dt.float16)
            nc.vector.tensor_tensor(dg16[:], gf32[:], gsh[:], ALU.subtract)

            # p-major iota: flat column i*J + j holds row value j + J*i, so
            # sub-tile slice [:, j*P:(j+1)*P] column i maps to psum
            # partition i = local row i*J + j.
            iota = cst.tile([S, chunk_rows], dt.float32)
            nc.gpsimd.iota(
                iota[:],
                pattern=[[1, J], [J, P]],
                base=0,
                channel_multiplier=0,
                allow_small_or_imprecise_dtypes=True,
            )

            # ---------------- main loop (software-pipelined) ----------------
            # Emission order per engine IS execution order (engines are FIFO).
            # Per iteration c the engine queues see (skew=2):
            #   Sync: in(c+pf), out(c-2)
            #   DVE : bn x8(c), mult(c-2), recip(c)
            #   ACT : norm x8(c-2), msq(c), sqrt(c)
            #   Pool: is_lt(c), combine(c), beta(c)
            #   PE  : matmul x8(c-2)
            # The skew-2 pipeline gives the bn->combine->sqrt->recip->beta
            # chain (which hops 4 engines) a full iteration of slack before
            # apply consumes alpha/beta, so no engine blocks on it.
            def stats_a(c):
                int_ = ints[c]
                sh = sgn_pool.tile([S, chunk_rows], dt.float16, tag="sh")
                nc.gpsimd.tensor_scalar(
                    sh[:], iota[:], ohi_sb[:, c : c + 1], None, op0=ALU.is_lt
                )
                st6 = sts_pool.tile([P, J, 6], dt.float32, tag="st6")
                for j in range(J):
                    nc.vector.bn_stats(st6[:, j, :], int_[:, j, :])
                return (int_, sh, st6)

            def stats_b(c, staged):
                int_, sh, st6 = staged
                m_e = st6[:, :, 1]
                cv_e = st6[:, :, 2]
                m_o = st6[:, :, 4]
                cv_o = st6[:, :, 5]

                msum = sts_pool.tile([P, J], dt.float32, tag="msum")
                nc.gpsimd.tensor_tensor(msum[:], m_e, m_o, ALU.add)
                mdif = sts_pool.tile([P, J], dt.float32, tag="mdif")
                nc.gpsimd.tensor_tensor(mdif[:], m_e, m_o, ALU.subtract)
                cvs = sts_pool.tile([P, J], dt.float32, tag="cvs")
                nc.gpsimd.tensor_tensor(cvs[:], cv_e, cv_o, ALU.add)
                negm = sts_pool.tile([P, J], dt.float32, tag="negm")
                nc.gpsimd.tensor_scalar(negm[:], msum[:], -0.5, None, op0=ALU.mult)
                msq = sts_pool.tile([P, J], dt.float32, tag="msq")
                nc.scalar.activation(msq[:], mdif[:], AF.Square, scale=0.5)
                v = sts_pool.tile([P, J], dt.float32, tag="v")
                nc.gpsimd.tensor_scalar(
                    v[:], cvs[:], 1.0 / D, EPS, op0=ALU.mult, op1=ALU.add
                )
                nc.gpsimd.tensor_tensor(v[:], v[:], msq[:], ALU.add)
                sdev = sts_pool.tile([P, J], dt.float32, tag="sdev")
                nc.scalar.activation(sdev[:], v[:], AF.Sqrt)
                alpha = sts_pool.tile([P, J], dt.float32, tag="alpha")
                nc.vector.reciprocal(alpha[:], sdev[:])
                beta = sts_pool.tile([P, J], dt.float32, tag="beta")
                nc.gpsimd.tensor_tensor(beta[:], negm[:], alpha[:], ALU.mult)
                return (int_, sh, alpha, beta)

            def apply_phase(c, staged):
                int_, sh, alpha, beta = staged
                ints.pop(c)
                ps = ps_pool.tile([P, J, D], dt.float32, tag="ps")
                for j in range(J):
                    nc.scalar.activation(
                        int_[:, j, :],
                        int_[:, j, :],
                        AF.Identity,
                        bias=beta[:, j : j + 1],
                        scale=alpha[:, j : j + 1],
                    )
                    nc.tensor.matmul(
                        ps[:, j, :],
                        sh[:, j * P : (j + 1) * P],
                        dg16[:],
                        start=True,
                        stop=True,
                    )
                nc.vector.tensor_tensor(int_[:], int_[:], ps[:], ALU.mult)
                nc.sync.dma_start(out_v[c], int_[:])

            staged = {}
            for c in range(nchunks):
                if c + prefetch < nchunks:
                    dma_in(c + prefetch)
                part = stats_a(c)
                if c >= skew:
                    apply_phase(c - skew, staged.pop(c - skew))
                staged[c] = stats_b(c, part)
            for c in range(nchunks - skew, nchunks):
                apply_phase(c, staged.pop(c))

    nc.compile()
    return nc


def make_in_maps(feat, global_feat, offset, W, b, n_cores=N_CORES, chunk_tiles=8):
    """Shard the full inputs into per-core in_maps."""
    N, D = feat.shape
    S = offset.shape[0]
    rows = N // n_cores
    chunk_rows = P * chunk_tiles
    nchunks = rows // chunk_rows
    feat = np.asarray(feat, dtype=np.float32)
    offset = np.asarray(offset, dtype=np.int64)
    gfT = np.ascontiguousarray(np.asarray(global_feat, dtype=np.float32).T)
    WT = np.ascontiguousarray(np.asarray(W, dtype=np.float32).T)
    b_ = np.asarray(b, dtype=np.float32).reshape(1, D)
    ends = offset
    in_maps = []
    for c in range(n_cores):
        base = c * rows
        hi_c = np.clip(ends - base, 0, rows).astype(np.float32)
        # off_hi[s, ch] = hi_s - ch*chunk_rows (is_lt thresholds per chunk)
        off_hi = hi_c[:, None] - (np.arange(nchunks, dtype=np.float32) * chunk_rows)[None, :]
        in_maps.append(
            {
                "feat": np.ascontiguousarray(feat[base : base + rows]),
                "gfT": gfT,
                "WT": WT,
                "b": b_,
                "off_hi": np.ascontiguousarray(off_hi.astype(np.float32)),
            }
        )
    return in_maps

_NC_CACHE = {}

last_exec_time_ns = None


def kernel(feat, global_feat, offset, W, b):
    """Full inputs in, full output out. Shards across 8 NeuronCores."""
    global last_exec_time_ns
    if "nc" not in _NC_CACHE:
        _NC_CACHE["nc"] = build_kernel(
            rows=ROWS,
            chunk_tiles=8,
            skew=2,
            prefetch=6,
            bufs_in=9,
        )
    nc = _NC_CACHE["nc"]
    in_maps = make_in_maps(feat, global_feat, offset, W, b, n_cores=N_CORES)
    kwargs = {}
    if os.environ.get("ADALN_TRACE") == "1":
        _install_profshim()
        import tempfile

        kwargs = {"trace": True, "tmpdir": tempfile.mkdtemp(prefix="adaln_prof_")}
    res = run_bass_kernel_spmd(nc, in_maps, core_ids=list(range(N_CORES)), **kwargs)
    last_exec_time_ns = res.exec_time_ns
    return np.concatenate([res.results[i]["out"] for i in range(N_CORES)], axis=0)


# revision 18
# speedup vs baseline: 1.7999x; 1.7999x over previous
"""AdaLayerNorm (ragged gather_csr + LayerNorm) Trainium2 Bass kernel.

Runs SPMD on 8 NeuronCores, data-parallel over the node dimension: each core
gets a contiguous 16384-row shard of `feat`, replicated affine weights, and
its segment end-offsets clipped to the local row range, so the gather_csr
expansion stays device-local (per the sharding hint).

Per-core graph, HBM-roofline oriented (the kernel is DMA-bound: 64 MB of
feat in/out per core at ~410 GB/s aggregate over 16 DMA engines):
- Chunked 2 MB DMAs in a p-major row layout: each of the 128 partitions holds
  8 consecutive rows, so every DMA descriptor moves one contiguous 16 KB span.
- Engine balance keeps every pointwise engine under the ~10.2 us/chunk DMA
  pace: LayerNorm stats via bn_stats + final multiply on DVE; stats-combine
  small-ops on GpSimd; segment mask + sqrt + normalize on ScalarE. (GPSIMD
  cannot touch PSUM and its is_lt is ~20x slower than DVE's, so the mask
  moved to ScalarE's Sign activation instead.)
- The per-node segment broadcast g[seg(i), :] is computed on-device as ONE
  accumulating TensorE matmul per 128-row tile against a +-1 comparison
  matrix S01[s, i] = sign(end_s - 0.5 - row_i) (one ScalarE Sign activation
  per chunk; Sign shares an act table with Identity/Square/Sqrt so there is
  no table-switch cost) and telescoped row differences dg[s] = g[s] - g[s+1]
  plus a 65th always-on row holding g[0]:
      sum_s S01[s,i]*dg[s] + g[0] = 2*g[seg(row_i)]        (exact)
  The factor 2 is folded into alpha/beta via Sqrt(4*(var+eps)).
  dg is fp16 (the only surviving rounding, ~2e-3 relative).
- Normalize and multiply are IN-PLACE on the input tile (no separate output
  buffer), freeing SBUF for a 9-deep input pool so the input DMA stream never
  stalls; outputs stream per half-chunk to cut the drain tail.
- Affine weight loads use a (p k) row interleave so every descriptor is a
  contiguous 8 KB / 1 KB span instead of 512 small strided rows.
"""

import sys

sys.path.insert(0, "/opt/trn_rl_repo")

import os

import numpy as np

import concourse.bacc as bacc
import concourse.mybir as mybir
import concourse.tile as tile
from concourse.bass_utils import run_bass_kernel_spmd

dt = mybir.dt
AF = mybir.ActivationFunctionType
ALU = mybir.AluOpType

EPS = 1e-5
P = 128
N_CORES = 8
N, D_FULL, S_FULL = 131072, 512, 64
ROWS = N // N_CORES


def _install_profshim():
    """Best-effort NTFF profiling hook for trace runs (optional)."""
    try:
        import types

        import antenv

        if getattr(antenv, "axon_hooks", None) is not None:
            return
        sys.path.insert(0, "/root/.axon_site/trn_agent_boot")
        import trn_boot

        hook = trn_boot._ntff_profile_via_ctypes("/opt/axon/libaxon_pjrt.so")
        m = types.ModuleType("antenv.axon_hooks")
        state = {"hook": hook}
        m.set_axon_ntff_profile_hook = lambda h: state.__setitem__("hook", h)
        m.get_axon_ntff_profile_hook = lambda: state["hook"]
        sys.modules["antenv.axon_hooks"] = m
        antenv.axon_hooks = m
    except Exception:
        pass


def build_kernel(
    rows=16384,
    D=512,
    S=64,
    chunk_tiles=8,
    skew=2,
    prefetch=6,
    bufs_in=9,
    bufs_sgn=5,
    bufs_sts=5,
):
    """Build the per-core Bass graph. All 8 cores run this same graph."""
    J = chunk_tiles
    chunk_rows = P * J
    nchunks = rows // chunk_rows
    assert rows % chunk_rows == 0
    WD = D
    KCH = WD // P

    S1 = S + 1
    nc = bacc.Bacc("TRN2", target_bir_lowering=False, debug=False)
    feat = nc.declare_dram_parameter("feat", [rows, D], dt.float32, isOutput=False)
    gfT = nc.declare_dram_parameter("gfT", [WD, S], dt.float32, isOutput=False)
    WT = nc.declare_dram_parameter("WT", [WD, D], dt.float32, isOutput=False)
    bb = nc.declare_dram_parameter("b", [1, D], dt.float32, isOutput=False)
    ohi = nc.declare_dram_parameter("off_hi", [S1, nchunks], dt.float32, isOutput=False)
    out = nc.declare_dram_parameter("out", [rows, D], dt.float32, isOutput=True)

    with tile.TileContext(nc) as tc:
        with (
            tc.tile_pool(name="const", bufs=1) as cst,
            tc.tile_pool(name="inb", bufs=bufs_in) as in_pool,
            tc.tile_pool(name="sgn", bufs=bufs_sgn) as sgn_pool,
            tc.tile_pool(name="sts", bufs=bufs_sts) as sts_pool,
            tc.tile_pool(name="ps", bufs=1, space="PSUM") as ps_pool,
        ):
            # First feat chunks queue ahead of all init DMAs (startup latency)
            feat_v = feat.ap().rearrange("(c p j) d -> c p j d", p=P, j=J)
            out_v = out.ap().rearrange("(c p j) d -> c p j d", p=P, j=J)
            ints = {}

            def dma_in(c):
                ints[c] = in_pool.tile(
                    [P, J, D], dt.float32, tag="int", name=f"int{c}"
                )
                nc.sync.dma_start(ints[c][:], feat_v[c])

            for c0 in range(min(prefetch, nchunks)):
                dma_in(c0)

            # ---------------- init: affine g = gf @ W.T + b ----------------
            # (p k) interleave: partition p holds WD-rows 4p..4p+3, one
            # contiguous span per partition per DMA descriptor.
            wt_sb = cst.tile([P, KCH, D], dt.float32)
            nc.sync.dma_start(wt_sb[:], WT.ap().rearrange("(p k) d -> p k d", k=KCH))
            gft_sb = cst.tile([P, KCH, S], dt.float32)
            nc.sync.dma_start(gft_sb[:], gfT.ap().rearrange("(p k) s -> p k s", k=KCH))
            b_sb = cst.tile([1, D], dt.float32)
            nc.sync.dma_start(b_sb[:], bb.ap())
            ohi_sb = cst.tile([S1, nchunks], dt.float32)
            nc.sync.dma_start(ohi_sb[:], ohi.ap())

            ones1 = cst.tile([1, S], dt.float32)
            nc.vector.memset(ones1[:], 1.0)

            psg = ps_pool.tile([S, D], dt.float32, tag="ps")
            for k in range(KCH):
                nc.tensor.matmul(
                    psg[:],
                    gft_sb[:, k, :],
                    wt_sb[:, k, :],
                    start=(k == 0),
                    stop=False,
                )
            nc.tensor.matmul(psg[:], ones1[:], b_sb[:], start=False, stop=True)
            # telescoped differences: dg[s] = g[s] - g[s+1]  (dg[63] = g[63]);
            # sum_{s >= seg} dg[s] telescopes exactly to g[seg]
            gf32 = cst.tile([S, D], dt.float32)
            nc.scalar.activation(gf32[:], psg[:], AF.Copy)
            # gext rows 0..S-1: dg[s] = g[s] - g[s+1]; row S: g[0]
            # (sum_s sign[s]*dg[s] + g[0] telescopes to 2*g[seg] exactly)
            gsh = cst.tile([S, D], dt.float32)
            nc.vector.memset(gsh[:], 0.0)
            nc.sync.dma_start(gsh[0 : S - 1, :], gf32[1:S, :])
            gext = cst.tile([S1, D], dt.float32)
            nc.vector.tensor_tensor(gext[0:S, :], gf32[:], gsh[:], ALU.subtract)
            nc.sync.dma_start(gext[S:S1, :], gf32[0:1, :])
            dg16 = cst.tile([S1, D], dt.float16)
            nc.vector.tensor_scalar(dg16[:], gext[:], 0.0, None, op0=ALU.add)

            # p-major iota: flat column i*J + j holds row value j + J*i, so
            # sub-tile slice [:, j*P:(j+1)*P] column i maps to psum
            # partition i = local row i*J + j.
            iota = cst.tile([S1, chunk_rows], dt.float32)
            nc.gpsimd.iota(
                iota[:],
                pattern=[[1, J], [J, P]],
                base=0,
                channel_multiplier=0,
                allow_small_or_imprecise_dtypes=True,
            )

            # ---------------- main loop (software-pipelined) ----------------
            # Emission order per engine IS execution order (engines are FIFO).
            # Per iteration c the engine queues see (skew=2):
            #   Sync: in(c+pf), out_a(c-2), out_b(c-2)
            #   DVE : bn x8(c), mult_a(c-2), mult_b(c-2), recip(c)
            #   ACT : sign(c), norm x8(c-2), msq(c), sqrt(c)
            #   Pool: combine(c), beta(c)
            #   PE  : matmul x8(c-2)
            # The skew-2 pipeline gives the bn->combine->sqrt->recip->beta
            # chain (which hops 4 engines) a full iteration of slack before
            # apply consumes alpha/beta; the half-chunk mult split lets DVE
            # start multiplying as soon as the first 4 norms land.
            def stats_a(c):
                int_ = ints[c]
                sh = sgn_pool.tile([S1, chunk_rows], dt.float16, tag="sh")
                nc.scalar.activation(
                    sh[:],
                    iota[:],
                    AF.Sign,
                    bias=ohi_sb[:, c : c + 1],
                    scale=-1.0,
                )
                st6 = sts_pool.tile([P, J, 6], dt.float32, tag="st6")
                for j in range(J):
                    nc.vector.bn_stats(st6[:, j, :], int_[:, j, :])
                return (int_, sh, st6)

            def stats_b(c, staged):
                int_, sh, st6 = staged
                m_e = st6[:, :, 1]
                cv_e = st6[:, :, 2]
                m_o = st6[:, :, 4]
                cv_o = st6[:, :, 5]

                msum = sts_pool.tile([P, J], dt.float32, tag="msum")
                nc.gpsimd.tensor_tensor(msum[:], m_e, m_o, ALU.add)
                mdif = sts_pool.tile([P, J], dt.float32, tag="mdif")
                nc.gpsimd.tensor_tensor(mdif[:], m_e, m_o, ALU.subtract)
                cvs = sts_pool.tile([P, J], dt.float32, tag="cvs")
                nc.gpsimd.tensor_tensor(cvs[:], cv_e, cv_o, ALU.add)
                negm = sts_pool.tile([P, J], dt.float32, tag="negm")
                nc.gpsimd.tensor_scalar(negm[:], msum[:], -0.5, None, op0=ALU.mult)
                msq = sts_pool.tile([P, J], dt.float32, tag="msq")
                nc.scalar.activation(msq[:], mdif[:], AF.Square, scale=0.5)
                v = sts_pool.tile([P, J], dt.float32, tag="v")
                nc.gpsimd.tensor_scalar(
                    v[:], cvs[:], 1.0 / D, EPS, op0=ALU.mult, op1=ALU.add
                )
                nc.gpsimd.tensor_tensor(v[:], v[:], msq[:], ALU.add)
                # Sqrt(4v) = 2*sdev: folds the ps = 2*g factor into alpha/beta
                sdev = sts_pool.tile([P, J], dt.float32, tag="sdev")
                nc.scalar.activation(sdev[:], v[:], AF.Sqrt, scale=4.0)
                alpha = sts_pool.tile([P, J], dt.float32, tag="alpha")
                nc.vector.reciprocal(alpha[:], sdev[:])
                beta = sts_pool.tile([P, J], dt.float32, tag="beta")
                nc.gpsimd.tensor_tensor(beta[:], negm[:], alpha[:], ALU.mult)
                return (int_, sh, alpha, beta)

            def apply_phase(c, staged):
                int_, sh, alpha, beta = staged
                ints.pop(c)
                ps = ps_pool.tile([P, J, D], dt.float32, tag="ps")
                H = J // 2
                for grp in (slice(0, H), slice(H, J)):
                    for j in range(grp.start, grp.stop):
                        nc.scalar.activation(
                            int_[:, j, :],
                            int_[:, j, :],
                            AF.Identity,
                            bias=beta[:, j : j + 1],
                            scale=alpha[:, j : j + 1],
                        )
                        nc.tensor.matmul(
                            ps[:, j, :],
                            sh[:, j * P : (j + 1) * P],
                            dg16[:],
                            start=True,
                            stop=True,
                        )
                    nc.vector.tensor_tensor(
                        int_[:, grp, :], int_[:, grp, :], ps[:, grp, :], ALU.mult
                    )
                    nc.sync.dma_start(out_v[c][:, grp, :], int_[:, grp, :])

            staged = {}
            for c in range(nchunks):
                if c + prefetch < nchunks:
                    dma_in(c + prefetch)
                part = stats_a(c)
                if c >= skew:
                    apply_phase(c - skew, staged.pop(c - skew))
                staged[c] = stats_b(c, part)
            for c in range(nchunks - skew, nchunks):
                apply_phase(c, staged.pop(c))

    nc.compile()
    return nc


def make_in_maps(feat, global_feat, offset, W, b, n_cores=N_CORES, chunk_tiles=8):
    """Shard the full inputs into per-core in_maps."""
    N, D = feat.shape
    S = offset.shape[0]
    rows = N // n_cores
    chunk_rows = P * chunk_tiles
    nchunks = rows // chunk_rows
    feat = np.asarray(feat, dtype=np.float32)
    offset = np.asarray(offset, dtype=np.int64)
    gfT = np.ascontiguousarray(np.asarray(global_feat, dtype=np.float32).T)
    WT = np.ascontiguousarray(np.asarray(W, dtype=np.float32).T)
    b_ = np.asarray(b, dtype=np.float32).reshape(1, D)
    ends = offset
    in_maps = []
    for c in range(n_cores):
        base = c * rows
        hi_c = np.clip(ends - base, 0, rows).astype(np.float32)
        # off_hi[s, ch] = hi_s - ch*chunk_rows - 0.5: Sign thresholds per
        # chunk; row S is a huge sentinel so its sign is always +1.
        off_hi = np.full((S + 1, nchunks), 1e9, dtype=np.float32)
        off_hi[:S, :] = (
            hi_c[:, None]
            - (np.arange(nchunks, dtype=np.float32) * chunk_rows)[None, :]
            - 0.5
        )
        in_maps.append(
            {
                "feat": np.ascontiguousarray(feat[base : base + rows]),
                "gfT": gfT,
                "WT": WT,
                "b": b_,
                "off_hi": np.ascontiguousarray(off_hi),
            }
        )
    return in_maps

_NC_CACHE = {}

last_exec_time_ns = None


def kernel(feat, global_feat, offset, W, b):
    """Full inputs in, full output out. Shards across 8 NeuronCores."""
    global last_exec_time_ns
    if "nc" not in _NC_CACHE:
        _NC_CACHE["nc"] = build_kernel(
            rows=ROWS,
            chunk_tiles=8,
            skew=2,
            prefetch=6,
            bufs_in=9,
        )
    nc = _NC_CACHE["nc"]
    in_maps = make_in_maps(feat, global_feat, offset, W, b, n_cores=N_CORES)
    kwargs = {}
    if os.environ.get("ADALN_TRACE") == "1":
        _install_profshim()
        import tempfile

        kwargs = {"trace": True, "tmpdir": tempfile.mkdtemp(prefix="adaln_prof_")}
    res = run_bass_kernel_spmd(nc, in_maps, core_ids=list(range(N_CORES)), **kwargs)
    last_exec_time_ns = res.exec_time_ns
    return np.concatenate([res.results[i]["out"] for i in range(N_CORES)], axis=0)


# revision 19
# speedup vs baseline: 1.8418x; 1.0233x over previous
"""AdaLayerNorm (ragged gather_csr + LayerNorm) Trainium2 Bass kernel.

Runs SPMD on 8 NeuronCores, data-parallel over the node dimension: each core
gets a contiguous 16384-row shard of `feat`, replicated affine weights, and
its segment end-offsets clipped to the local row range, so the gather_csr
expansion stays device-local (per the sharding hint).

Per-core graph, HBM-roofline oriented (the kernel is DMA-bound: 64 MB of
feat in/out per core at ~410 GB/s aggregate over 16 DMA engines):
- Chunked 2 MB DMAs in a p-major row layout: each of the 128 partitions holds
  8 consecutive rows, so every DMA descriptor moves one contiguous 16 KB span.
- Engine balance keeps every pointwise engine under the ~10.2 us/chunk DMA
  pace: LayerNorm stats via bn_stats + final multiply on DVE; stats-combine
  small-ops on GpSimd; segment mask + sqrt + normalize on ScalarE. (GPSIMD
  cannot touch PSUM and its is_lt is ~20x slower than DVE's, so the mask
  moved to ScalarE's Sign activation instead.)
- The per-node segment broadcast g[seg(i), :] is computed on-device as ONE
  accumulating TensorE matmul per 128-row tile against a +-1 comparison
  matrix S01[s, i] = sign(end_s - 0.5 - row_i) (one ScalarE Sign activation
  per chunk; Sign shares an act table with Identity/Square/Sqrt so there is
  no table-switch cost) and telescoped row differences dg[s] = g[s] - g[s+1]
  plus a 65th always-on row holding g[0]:
      sum_s S01[s,i]*dg[s] + g[0] = 2*g[seg(row_i)]        (exact)
  The factor 2 is folded into alpha/beta via Sqrt(4*(var+eps)).
  dg is fp16 (the only surviving rounding, ~2e-3 relative).
- Normalize and multiply are IN-PLACE on the input tile (no separate output
  buffer), freeing SBUF for a 9-deep input pool so the input DMA stream never
  stalls; outputs stream per half-chunk to cut the drain tail.
- Affine weight loads use a (p k) row interleave so every descriptor is a
  contiguous 8 KB / 1 KB span instead of 512 small strided rows.
"""

import sys

sys.path.insert(0, "/opt/trn_rl_repo")

import os

import numpy as np

import concourse.bacc as bacc
import concourse.mybir as mybir
import concourse.tile as tile
from concourse.bass_utils import run_bass_kernel_spmd

dt = mybir.dt
AF = mybir.ActivationFunctionType
ALU = mybir.AluOpType

EPS = 1e-5
P = 128
N_CORES = 8
N, D_FULL, S_FULL = 131072, 512, 64
ROWS = N // N_CORES


def _install_profshim():
    """Best-effort NTFF profiling hook for trace runs (optional)."""
    try:
        import types

        import antenv

        if getattr(antenv, "axon_hooks", None) is not None:
            return
        sys.path.insert(0, "/root/.axon_site/trn_agent_boot")
        import trn_boot

        hook = trn_boot._ntff_profile_via_ctypes("/opt/axon/libaxon_pjrt.so")
        m = types.ModuleType("antenv.axon_hooks")
        state = {"hook": hook}
        m.set_axon_ntff_profile_hook = lambda h: state.__setitem__("hook", h)
        m.get_axon_ntff_profile_hook = lambda: state["hook"]
        sys.modules["antenv.axon_hooks"] = m
        antenv.axon_hooks = m
    except Exception:
        pass


def build_kernel(
    rows=16384,
    D=512,
    S=64,
    chunk_tiles=8,
    skew=2,
    prefetch=6,
    bufs_in=9,
    bufs_sgn=5,
    bufs_sts=5,
):
    """Build the per-core Bass graph. All 8 cores run this same graph."""
    J = chunk_tiles
    chunk_rows = P * J
    nchunks = rows // chunk_rows
    assert rows % chunk_rows == 0
    WD = D
    KCH = WD // P

    S1 = S + 1
    nc = bacc.Bacc("TRN2", target_bir_lowering=False, debug=False)
    feat = nc.declare_dram_parameter("feat", [rows, D], dt.float32, isOutput=False)
    gfT = nc.declare_dram_parameter("gfT", [WD, S], dt.float32, isOutput=False)
    WT = nc.declare_dram_parameter("WT", [WD, D], dt.float32, isOutput=False)
    bb = nc.declare_dram_parameter("b", [1, D], dt.float32, isOutput=False)
    ohi = nc.declare_dram_parameter("off_hi", [S1, nchunks], dt.float32, isOutput=False)
    out = nc.declare_dram_parameter("out", [rows, D], dt.float32, isOutput=True)

    with tile.TileContext(nc) as tc:
        with (
            tc.tile_pool(name="const", bufs=1) as cst,
            tc.tile_pool(name="inb", bufs=bufs_in) as in_pool,
            tc.tile_pool(name="sgn", bufs=bufs_sgn) as sgn_pool,
            tc.tile_pool(name="sts", bufs=bufs_sts) as sts_pool,
            tc.tile_pool(name="ps", bufs=1, space="PSUM") as ps_pool,
        ):
            # First feat chunks queue ahead of all init DMAs (startup latency)
            feat_v = feat.ap().rearrange("(c p j) d -> c p j d", p=P, j=J)
            out_v = out.ap().rearrange("(c p j) d -> c p j d", p=P, j=J)
            ints = {}

            def dma_in(c):
                ints[c] = in_pool.tile(
                    [P, J, D], dt.float32, tag="int", name=f"int{c}"
                )
                nc.sync.dma_start(ints[c][:], feat_v[c])

            for c0 in range(min(prefetch, nchunks)):
                dma_in(c0)

            # ---------------- init: affine g = gf @ W.T + b ----------------
            # (p k) interleave: partition p holds WD-rows 4p..4p+3, one
            # contiguous span per partition per DMA descriptor.
            wt_sb = cst.tile([P, KCH, D], dt.float32)
            nc.sync.dma_start(wt_sb[:], WT.ap().rearrange("(p k) d -> p k d", k=KCH))
            gft_sb = cst.tile([P, KCH, S], dt.float32)
            nc.sync.dma_start(gft_sb[:], gfT.ap().rearrange("(p k) s -> p k s", k=KCH))
            b_sb = cst.tile([1, D], dt.float32)
            nc.sync.dma_start(b_sb[:], bb.ap())
            ohi_sb = cst.tile([S1, nchunks], dt.float32)
            nc.sync.dma_start(ohi_sb[:], ohi.ap())

            ones1 = cst.tile([1, S], dt.float32)
            nc.vector.memset(ones1[:], 1.0)

            psg = ps_pool.tile([S, D], dt.float32, tag="ps")
            for k in range(KCH):
                nc.tensor.matmul(
                    psg[:],
                    gft_sb[:, k, :],
                    wt_sb[:, k, :],
                    start=(k == 0),
                    stop=False,
                )
            nc.tensor.matmul(psg[:], ones1[:], b_sb[:], start=False, stop=True)
            # telescoped differences: dg[s] = g[s] - g[s+1]  (dg[63] = g[63]);
            # sum_{s >= seg} dg[s] telescopes exactly to g[seg]
            gf32 = cst.tile([S, D], dt.float32)
            nc.scalar.activation(gf32[:], psg[:], AF.Copy)
            # gext rows 0..S-1: dg[s] = g[s] - g[s+1]; row S: g[0]
            # (sum_s sign[s]*dg[s] + g[0] telescopes to 2*g[seg] exactly)
            gsh = cst.tile([S, D], dt.float32)
            nc.vector.memset(gsh[:], 0.0)
            nc.sync.dma_start(gsh[0 : S - 1, :], gf32[1:S, :])
            gext = cst.tile([S1, D], dt.float32)
            nc.vector.tensor_tensor(gext[0:S, :], gf32[:], gsh[:], ALU.subtract)
            nc.sync.dma_start(gext[S:S1, :], gf32[0:1, :])
            dg16 = cst.tile([S1, D], dt.float16)
            nc.vector.tensor_scalar(dg16[:], gext[:], 0.0, None, op0=ALU.add)

            # p-major iota: flat column i*J + j holds row value j + J*i, so
            # sub-tile slice [:, j*P:(j+1)*P] column i maps to psum
            # partition i = local row i*J + j.
            iota = cst.tile([S1, chunk_rows], dt.float32)
            nc.gpsimd.iota(
                iota[:],
                pattern=[[1, J], [J, P]],
                base=0,
                channel_multiplier=0,
                allow_small_or_imprecise_dtypes=True,
            )

            # ---------------- main loop (software-pipelined) ----------------
            # Emission order per engine IS execution order (engines are FIFO).
            # Per iteration c the engine queues see (skew=2):
            #   Sync: in(c+pf), out_a(c-2), out_b(c-2)
            #   DVE : bn x8(c), mult_a(c-2), mult_b(c-2), recip(c)
            #   ACT : sign(c), norm x8(c-2), msq(c), sqrt(c)
            #   Pool: combine(c), beta(c)
            #   PE  : matmul x8(c-2)
            # The skew-2 pipeline gives the bn->combine->sqrt->recip->beta
            # chain (which hops 4 engines) a full iteration of slack before
            # apply consumes alpha/beta; the half-chunk mult split lets DVE
            # start multiplying as soon as the first 4 norms land.
            def stats_a(c):
                int_ = ints[c]
                sh = sgn_pool.tile([S1, chunk_rows], dt.float16, tag="sh")
                nc.scalar.activation(
                    sh[:],
                    iota[:],
                    AF.Sign,
                    bias=ohi_sb[:, c : c + 1],
                    scale=-1.0,
                )
                st6 = sts_pool.tile([P, J, 6], dt.float32, tag="st6")
                for j in range(J):
                    nc.vector.bn_stats(st6[:, j, :], int_[:, j, :])
                return (int_, sh, st6)

            def stats_b(c, staged):
                int_, sh, st6 = staged
                m_e = st6[:, :, 1]
                cv_e = st6[:, :, 2]
                m_o = st6[:, :, 4]
                cv_o = st6[:, :, 5]

                msum = sts_pool.tile([P, J], dt.float32, tag="msum")
                nc.gpsimd.tensor_tensor(msum[:], m_e, m_o, ALU.add)
                mdif = sts_pool.tile([P, J], dt.float32, tag="mdif")
                nc.gpsimd.tensor_tensor(mdif[:], m_e, m_o, ALU.subtract)
                cvs = sts_pool.tile([P, J], dt.float32, tag="cvs")
                nc.gpsimd.tensor_tensor(cvs[:], cv_e, cv_o, ALU.add)
                negm = sts_pool.tile([P, J], dt.float32, tag="negm")
                nc.gpsimd.tensor_scalar(negm[:], msum[:], -0.5, None, op0=ALU.mult)
                msq = sts_pool.tile([P, J], dt.float32, tag="msq")
                nc.scalar.activation(msq[:], mdif[:], AF.Square, scale=0.5)
                v = sts_pool.tile([P, J], dt.float32, tag="v")
                nc.gpsimd.tensor_scalar(
                    v[:], cvs[:], 1.0 / D, EPS, op0=ALU.mult, op1=ALU.add
                )
                nc.gpsimd.tensor_tensor(v[:], v[:], msq[:], ALU.add)
                # Sqrt(4v) = 2*sdev: folds the ps = 2*g factor into alpha/beta
                sdev = sts_pool.tile([P, J], dt.float32, tag="sdev")
                nc.scalar.activation(sdev[:], v[:], AF.Sqrt, scale=4.0)
                alpha = sts_pool.tile([P, J], dt.float32, tag="alpha")
                nc.vector.reciprocal(alpha[:], sdev[:])
                beta = sts_pool.tile([P, J], dt.float32, tag="beta")
                nc.gpsimd.tensor_tensor(beta[:], negm[:], alpha[:], ALU.mult)
                return (int_, sh, alpha, beta)

            def apply_phase(c, staged):
                int_, sh, alpha, beta = staged
                ints.pop(c)
                ps = ps_pool.tile([P, J, D], dt.float32, tag="ps")
                H = J // 2
                for grp in (slice(0, H), slice(H, J)):
                    for j in range(grp.start, grp.stop):
                        nc.scalar.activation(
                            int_[:, j, :],
                            int_[:, j, :],
                            AF.Identity,
                            bias=beta[:, j : j + 1],
                            scale=alpha[:, j : j + 1],
                        )
                        nc.tensor.matmul(
                            ps[:, j, :],
                            sh[:, j * P : (j + 1) * P],
                            dg16[:],
                            start=True,
                            stop=True,
                        )
                    nc.vector.tensor_tensor(
                        int_[:, grp, :], int_[:, grp, :], ps[:, grp, :], ALU.mult
                    )
                    # outs ride GpSimd's SWDGE queue: a separate descriptor
                    # ring from Sync's input queue, so input dma_starts that
                    # block on buffer-free semaphores can't delay them.
                    nc.gpsimd.dma_start(out_v[c][:, grp, :], int_[:, grp, :])

            staged = {}
            for c in range(nchunks):
                if c + prefetch < nchunks:
                    dma_in(c + prefetch)
                part = stats_a(c)
                if c >= skew:
                    apply_phase(c - skew, staged.pop(c - skew))
                staged[c] = stats_b(c, part)
            for c in range(nchunks - skew, nchunks):
                apply_phase(c, staged.pop(c))

    nc.compile()
    return nc


def make_in_maps(feat, global_feat, offset, W, b, n_cores=N_CORES, chunk_tiles=8):
    """Shard the full inputs into per-core in_maps."""
    N, D = feat.shape
    S = offset.shape[0]
    rows = N // n_cores
    chunk_rows = P * chunk_tiles
    nchunks = rows // chunk_rows
    feat = np.asarray(feat, dtype=np.float32)
    offset = np.asarray(offset, dtype=np.int64)
    gfT = np.ascontiguousarray(np.asarray(global_feat, dtype=np.float32).T)
    WT = np.ascontiguousarray(np.asarray(W, dtype=np.float32).T)
    b_ = np.asarray(b, dtype=np.float32).reshape(1, D)
    ends = offset
    in_maps = []
    for c in range(n_cores):
        base = c * rows
        hi_c = np.clip(ends - base, 0, rows).astype(np.float32)
        # off_hi[s, ch] = hi_s - ch*chunk_rows - 0.5: Sign thresholds per
        # chunk; row S is a huge sentinel so its sign is always +1.
        off_hi = np.full((S + 1, nchunks), 1e9, dtype=np.float32)
        off_hi[:S, :] = (
            hi_c[:, None]
            - (np.arange(nchunks, dtype=np.float32) * chunk_rows)[None, :]
            - 0.5
        )
        in_maps.append(
            {
                "feat": np.ascontiguousarray(feat[base : base + rows]),
                "gfT": gfT,
                "WT": WT,
                "b": b_,
                "off_hi": np.ascontiguousarray(off_hi),
            }
        )
    return in_maps

_NC_CACHE = {}

last_exec_time_ns = None


def kernel(feat, global_feat, offset, W, b):
    """Full inputs in, full output out. Shards across 8 NeuronCores."""
    global last_exec_time_ns
    if "nc" not in _NC_CACHE:
        _NC_CACHE["nc"] = build_kernel(
            rows=ROWS,
            chunk_tiles=8,
            skew=2,
            prefetch=6,
            bufs_in=9,
        )
    nc = _NC_CACHE["nc"]
    in_maps = make_in_maps(feat, global_feat, offset, W, b, n_cores=N_CORES)
    kwargs = {}
    if os.environ.get("ADALN_TRACE") == "1":
        _install_profshim()
        import tempfile

        kwargs = {"trace": True, "tmpdir": tempfile.mkdtemp(prefix="adaln_prof_")}
    res = run_bass_kernel_spmd(nc, in_maps, core_ids=list(range(N_CORES)), **kwargs)
    last_exec_time_ns = res.exec_time_ns
    return np.concatenate([res.results[i]["out"] for i in range(N_CORES)], axis=0)


# revision 25
# speedup vs baseline: 1.9195x; 1.0422x over previous
"""AdaLayerNorm (ragged gather_csr + LayerNorm) Trainium2 Bass kernel.

Runs SPMD on 8 NeuronCores, data-parallel over the node dimension: each core
gets a contiguous 16384-row shard of `feat`, replicated affine weights, and
its segment end-offsets clipped to the local row range, so the gather_csr
expansion stays device-local (per the sharding hint).

Per-core graph, HBM-roofline oriented (the kernel is DMA-bound: 64 MB of
feat in/out per core at ~410 GB/s aggregate over 16 DMA engines):
- Chunked 2 MB DMAs in a p-major row layout: each of the 128 partitions holds
  8 consecutive rows, so every DMA descriptor moves one contiguous 16 KB span.
- Engine balance keeps every pointwise engine under the ~10.2 us/chunk DMA
  pace: LayerNorm stats via bn_stats + final multiply on DVE; stats-combine
  small-ops on GpSimd; segment mask + sqrt + normalize on ScalarE. (GPSIMD
  cannot touch PSUM and its is_lt is ~20x slower than DVE's, so the mask
  moved to ScalarE's Sign activation instead.)
- The per-node segment broadcast g[seg(i), :] is computed on-device as ONE
  accumulating TensorE matmul per 128-row tile against a +-1 comparison
  matrix S01[s, i] = sign(end_s - 0.5 - row_i) (one ScalarE Sign activation
  per chunk; Sign shares an act table with Identity/Square/Sqrt so there is
  no table-switch cost) and telescoped row differences dg[s] = g[s] - g[s+1]
  plus a 65th always-on row holding g[0]:
      sum_s S01[s,i]*dg[s] + g[0] = 2*g[seg(row_i)]        (exact)
  The factor 2 is folded into alpha/beta via Sqrt(4*(var+eps)).
  dg is fp16 (the only surviving rounding, ~2e-3 relative).
- Normalize and multiply are IN-PLACE on the input tile (no separate output
  buffer), freeing SBUF for a 9-deep input pool so the input DMA stream never
  stalls; outputs stream per half-chunk to cut the drain tail.
- Affine weight loads use a (p k) row interleave so every descriptor is a
  contiguous 8 KB / 1 KB span instead of 512 small strided rows.
"""

import sys

sys.path.insert(0, "/opt/trn_rl_repo")

import os

import numpy as np

import concourse.bacc as bacc
import concourse.mybir as mybir
import concourse.tile as tile
from concourse.bass_utils import run_bass_kernel_spmd
from concourse.tile_rust import add_dep_helper


def _after(later, earlier, why):
    """Ordering-only edge: schedule `later` after `earlier` on its engine."""
    if later is not None and earlier is not None:
        add_dep_helper(later.ins, earlier.ins, sync=False, reason=why)

dt = mybir.dt
AF = mybir.ActivationFunctionType
ALU = mybir.AluOpType

EPS = 1e-5
P = 128
N_CORES = 8
N, D_FULL, S_FULL = 131072, 512, 64
ROWS = N // N_CORES


def _install_profshim():
    """Best-effort NTFF profiling hook for trace runs (optional)."""
    try:
        import types

        import antenv

        if getattr(antenv, "axon_hooks", None) is not None:
            return
        sys.path.insert(0, "/root/.axon_site/trn_agent_boot")
        import trn_boot

        hook = trn_boot._ntff_profile_via_ctypes("/opt/axon/libaxon_pjrt.so")
        m = types.ModuleType("antenv.axon_hooks")
        state = {"hook": hook}
        m.set_axon_ntff_profile_hook = lambda h: state.__setitem__("hook", h)
        m.get_axon_ntff_profile_hook = lambda: state["hook"]
        sys.modules["antenv.axon_hooks"] = m
        antenv.axon_hooks = m
    except Exception:
        pass


def build_kernel(
    rows=16384,
    D=512,
    S=64,
    chunk_tiles=8,
    skew=2,
    prefetch=6,
    bufs_in=9,
    bufs_sgn=5,
    bufs_sts=5,
):
    """Build the per-core Bass graph. All 8 cores run this same graph."""
    J = chunk_tiles
    chunk_rows = P * J
    nchunks = rows // chunk_rows
    assert rows % chunk_rows == 0
    WD = D
    KCH = WD // P

    S1 = S + 1
    nc = bacc.Bacc("TRN2", target_bir_lowering=False, debug=False)
    feat = nc.declare_dram_parameter("feat", [rows, D], dt.float32, isOutput=False)
    gfT = nc.declare_dram_parameter("gfT", [WD, S], dt.float32, isOutput=False)
    WT = nc.declare_dram_parameter("WT", [WD, D], dt.float32, isOutput=False)
    bb = nc.declare_dram_parameter("b", [1, D], dt.float32, isOutput=False)
    ohi = nc.declare_dram_parameter("off_hi", [S1, nchunks], dt.float32, isOutput=False)
    out = nc.declare_dram_parameter("out", [rows, D], dt.float32, isOutput=True)

    with tile.TileContext(nc) as tc:
        with (
            tc.tile_pool(name="const", bufs=1) as cst,
            tc.tile_pool(name="inb", bufs=bufs_in) as in_pool,
            tc.tile_pool(name="sgn", bufs=bufs_sgn) as sgn_pool,
            tc.tile_pool(name="sts", bufs=bufs_sts) as sts_pool,
            tc.tile_pool(name="ps", bufs=1, space="PSUM") as ps_pool,
        ):
            # First feat chunks queue ahead of all init DMAs (startup latency)
            feat_v = feat.ap().rearrange("(c p j) d -> c p j d", p=P, j=J)
            out_v = out.ap().rearrange("(c p j) d -> c p j d", p=P, j=J)
            ints = {}

            def dma_in(c):
                ints[c] = in_pool.tile(
                    [P, J, D], dt.float32, tag="int", name=f"int{c}"
                )
                nc.sync.dma_start(ints[c][:], feat_v[c])

            for c0 in range(min(prefetch, nchunks)):
                dma_in(c0)

            # ---------------- init: affine g = gf @ W.T + b ----------------
            # (p k) interleave: partition p holds WD-rows 4p..4p+3, one
            # contiguous span per partition per DMA descriptor.
            wt_sb = cst.tile([P, KCH, D], dt.float32)
            nc.sync.dma_start(wt_sb[:], WT.ap().rearrange("(p k) d -> p k d", k=KCH))
            gft_sb = cst.tile([P, KCH, S], dt.float32)
            nc.sync.dma_start(gft_sb[:], gfT.ap().rearrange("(p k) s -> p k s", k=KCH))
            b_sb = cst.tile([1, D], dt.float32)
            nc.sync.dma_start(b_sb[:], bb.ap())
            ohi_sb = cst.tile([S1, nchunks], dt.float32)
            nc.sync.dma_start(ohi_sb[:], ohi.ap())

            ones1 = cst.tile([1, S], dt.float32)
            nc.vector.memset(ones1[:], 1.0)

            psg = ps_pool.tile([S, D], dt.float32, tag="ps")
            for k in range(KCH):
                nc.tensor.matmul(
                    psg[:],
                    gft_sb[:, k, :],
                    wt_sb[:, k, :],
                    start=(k == 0),
                    stop=False,
                )
            nc.tensor.matmul(psg[:], ones1[:], b_sb[:], start=False, stop=True)
            # telescoped differences: dg[s] = g[s] - g[s+1]  (dg[63] = g[63]);
            # sum_{s >= seg} dg[s] telescopes exactly to g[seg]
            gf32 = cst.tile([S, D], dt.float32)
            nc.scalar.activation(gf32[:], psg[:], AF.Copy)
            # gext rows 0..S-1: dg[s] = g[s] - g[s+1]; row S: g[0]
            # (sum_s sign[s]*dg[s] + g[0] telescopes to 2*g[seg] exactly)
            gsh = cst.tile([S, D], dt.float32)
            nc.vector.memset(gsh[:], 0.0)
            nc.sync.dma_start(gsh[0 : S - 1, :], gf32[1:S, :])
            gext = cst.tile([S1, D], dt.float32)
            nc.vector.tensor_tensor(gext[0:S, :], gf32[:], gsh[:], ALU.subtract)
            nc.sync.dma_start(gext[S:S1, :], gf32[0:1, :])
            dg16 = cst.tile([S1, D], dt.float16)
            nc.vector.tensor_scalar(dg16[:], gext[:], 0.0, None, op0=ALU.add)

            # p-major iota: flat column i*J + j holds row value j + J*i, so
            # sub-tile slice [:, j*P:(j+1)*P] column i maps to psum
            # partition i = local row i*J + j.
            iota = cst.tile([S1, chunk_rows], dt.float32)
            nc.gpsimd.iota(
                iota[:],
                pattern=[[1, J], [J, P]],
                base=0,
                channel_multiplier=0,
                allow_small_or_imprecise_dtypes=True,
            )

            # ---------------- main loop (software-pipelined) ----------------
            # Emission order per engine IS execution order (engines are FIFO).
            # Per iteration c the engine queues see (skew=2):
            #   Sync: in(c+pf), out_a(c-2), out_b(c-2)
            #   DVE : bn x8(c), mult_a(c-2), mult_b(c-2), recip(c)
            #   ACT : sign(c), norm x8(c-2), msq(c), sqrt(c)
            #   Pool: combine(c), beta(c)
            #   PE  : matmul x8(c-2)
            # The skew-2 pipeline gives the bn->combine->sqrt->recip->beta
            # chain (which hops 4 engines) a full iteration of slack before
            # apply consumes alpha/beta; the half-chunk mult split lets DVE
            # start multiplying as soon as the first 4 norms land.
            def stats_a(c, prev_h):
                int_ = ints[c]
                sh = sgn_pool.tile([S1, chunk_rows], dt.float16, tag="sh")
                sign_i = nc.scalar.activation(
                    sh[:],
                    iota[:],
                    AF.Sign,
                    bias=ohi_sb[:, c : c + 1],
                    scale=-1.0,
                )
                _after(sign_i, prev_h.get("sqrt"), "ACT iter chain")
                st6 = sts_pool.tile([P, J, 6], dt.float32, tag="st6")
                bn_i = []
                for j in range(J):
                    bn_i.append(nc.vector.bn_stats(st6[:, j, :], int_[:, j, :]))
                _after(bn_i[0], prev_h.get("recip"), "DVE iter chain")
                return (int_, sh, st6), {"sign": sign_i, "bn_last": bn_i[-1]}

            def stats_b(c, staged, cur_h):
                int_, sh, st6 = staged
                m_e = st6[:, :, 1]
                cv_e = st6[:, :, 2]
                m_o = st6[:, :, 4]
                cv_o = st6[:, :, 5]

                msum = sts_pool.tile([P, J], dt.float32, tag="msum")
                nc.gpsimd.tensor_tensor(msum[:], m_e, m_o, ALU.add)
                mdif = sts_pool.tile([P, J], dt.float32, tag="mdif")
                nc.gpsimd.tensor_tensor(mdif[:], m_e, m_o, ALU.subtract)
                cvs = sts_pool.tile([P, J], dt.float32, tag="cvs")
                nc.gpsimd.tensor_tensor(cvs[:], cv_e, cv_o, ALU.add)
                negm = sts_pool.tile([P, J], dt.float32, tag="negm")
                nc.gpsimd.tensor_scalar(negm[:], msum[:], -0.5, None, op0=ALU.mult)
                msq = sts_pool.tile([P, J], dt.float32, tag="msq")
                msq_i = nc.scalar.activation(msq[:], mdif[:], AF.Square, scale=0.5)
                _after(msq_i, cur_h.get("norm_last"), "ACT stats-tail after norms")
                v = sts_pool.tile([P, J], dt.float32, tag="v")
                nc.gpsimd.tensor_scalar(
                    v[:], cvs[:], 1.0 / D, EPS, op0=ALU.mult, op1=ALU.add
                )
                nc.gpsimd.tensor_tensor(v[:], v[:], msq[:], ALU.add)
                # Sqrt(4v) = 2*sdev: folds the ps = 2*g factor into alpha/beta
                sdev = sts_pool.tile([P, J], dt.float32, tag="sdev")
                sqrt_i = nc.scalar.activation(sdev[:], v[:], AF.Sqrt, scale=4.0)
                alpha = sts_pool.tile([P, J], dt.float32, tag="alpha")
                recip_i = nc.vector.reciprocal(alpha[:], sdev[:])
                _after(recip_i, cur_h.get("mult_last"), "DVE recip after mults")
                beta = sts_pool.tile([P, J], dt.float32, tag="beta")
                nc.gpsimd.tensor_tensor(beta[:], negm[:], alpha[:], ALU.mult)
                cur_h["sqrt"] = sqrt_i
                cur_h["recip"] = recip_i
                return (int_, sh, alpha, beta)

            def apply_phase(c, staged, cur_h):
                int_, sh, alpha, beta = staged
                ints.pop(c)
                ps = ps_pool.tile([P, J, D], dt.float32, tag="ps")
                H = J // 2
                first_norm = True
                for grp in (slice(0, H), slice(H, J)):
                    for j in range(grp.start, grp.stop):
                        norm_i = nc.scalar.activation(
                            int_[:, j, :],
                            int_[:, j, :],
                            AF.Identity,
                            bias=beta[:, j : j + 1],
                            scale=alpha[:, j : j + 1],
                        )
                        if first_norm:
                            _after(norm_i, cur_h.get("sign"), "ACT norms after sign")
                            first_norm = False
                        nc.tensor.matmul(
                            ps[:, j, :],
                            sh[:, j * P : (j + 1) * P],
                            dg16[:],
                            start=True,
                            stop=True,
                        )
                    mult_i = nc.vector.tensor_tensor(
                        int_[:, grp, :], int_[:, grp, :], ps[:, grp, :], ALU.mult
                    )
                    if grp.start == 0:
                        _after(mult_i, cur_h.get("bn_last"), "DVE mults after bn batch")
                    cur_h["mult_last"] = mult_i
                    # outs ride GpSimd's SWDGE queue: a separate descriptor
                    # ring from Sync's input queue, so input dma_starts that
                    # block on buffer-free semaphores can't delay them.
                    nc.gpsimd.dma_start(out_v[c][:, grp, :], int_[:, grp, :])
                cur_h["norm_last"] = norm_i

            staged = {}
            prev_h = {}
            for c in range(nchunks):
                if c + prefetch < nchunks:
                    dma_in(c + prefetch)
                part, cur_h = stats_a(c, prev_h)
                if c >= skew:
                    apply_phase(c - skew, staged.pop(c - skew), cur_h)
                staged[c] = stats_b(c, part, cur_h)
                prev_h = cur_h
            for c in range(nchunks - skew, nchunks):
                apply_phase(c, staged.pop(c), {})

    nc.compile()
    return nc


def make_in_maps(feat, global_feat, offset, W, b, n_cores=N_CORES, chunk_tiles=8):
    """Shard the full inputs into per-core in_maps."""
    N, D = feat.shape
    S = offset.shape[0]
    rows = N // n_cores
    chunk_rows = P * chunk_tiles
    nchunks = rows // chunk_rows
    feat = np.asarray(feat, dtype=np.float32)
    offset = np.asarray(offset, dtype=np.int64)
    gfT = np.ascontiguousarray(np.asarray(global_feat, dtype=np.float32).T)
    WT = np.ascontiguousarray(np.asarray(W, dtype=np.float32).T)
    b_ = np.asarray(b, dtype=np.float32).reshape(1, D)
    ends = offset
    in_maps = []
    for c in range(n_cores):
        base = c * rows
        hi_c = np.clip(ends - base, 0, rows).astype(np.float32)
        # off_hi[s, ch] = hi_s - ch*chunk_rows - 0.5: Sign thresholds per
        # chunk; row S is a huge sentinel so its sign is always +1.
        off_hi = np.full((S + 1, nchunks), 1e9, dtype=np.float32)
        off_hi[:S, :] = (
            hi_c[:, None]
            - (np.arange(nchunks, dtype=np.float32) * chunk_rows)[None, :]
            - 0.5
        )
        in_maps.append(
            {
                "feat": np.ascontiguousarray(feat[base : base + rows]),
                "gfT": gfT,
                "WT": WT,
                "b": b_,
                "off_hi": np.ascontiguousarray(off_hi),
            }
        )
    return in_maps

_NC_CACHE = {}

last_exec_time_ns = None


def kernel(feat, global_feat, offset, W, b):
    """Full inputs in, full output out. Shards across 8 NeuronCores."""
    global last_exec_time_ns
    if "nc" not in _NC_CACHE:
        _NC_CACHE["nc"] = build_kernel(
            rows=ROWS,
            chunk_tiles=8,
            skew=2,
            prefetch=6,
            bufs_in=9,
        )
    nc = _NC_CACHE["nc"]
    in_maps = make_in_maps(feat, global_feat, offset, W, b, n_cores=N_CORES)
    kwargs = {}
    if os.environ.get("ADALN_TRACE") == "1":
        _install_profshim()
        import tempfile

        kwargs = {"trace": True, "tmpdir": tempfile.mkdtemp(prefix="adaln_prof_")}
    res = run_bass_kernel_spmd(nc, in_maps, core_ids=list(range(N_CORES)), **kwargs)
    last_exec_time_ns = res.exec_time_ns
    return np.concatenate([res.results[i]["out"] for i in range(N_CORES)], axis=0)
